# revision 13
# baseline (speedup 1.0000x reference)
"""MinGRU (parallel log-space scan) Trainium2 Bass kernel.

Problem (hardcoded):
    x:    [B=8, S=4096, D=1024] f32
    W_hg: [D=1024, 2*D=2048]    f32
    out:  [B=8, S=4096, D=1024] f32

    hg = x @ W_hg ; hidden, gate = split(hg)
    h_t = (1-z_t) * h_{t-1} + z_t * g(hidden_t),  z = sigmoid(gate),
    g(v) = v + 0.5 if v >= 0 else sigmoid(v)  ==  max(v + 0.5, sigmoid(v))

Sharding: data-parallel over batch, one batch row per NeuronCore (8 cores),
W_hg replicated.

Layout strategy: the scan must run along the free dimension (channels on
partitions), so the device works entirely in the transposed layout
hg^T/h^T = [channels, seq]. The host passes x pre-transposed per batch row
and transposes the returned h^T back, so the device does no layout
conversion at all — the PE runs only the projection matmuls, ACT runs the
sigmoids, and the DVE runs the fused pointwise ops plus the native
tensor_tensor_scan linear recurrence.

The projection runs in bf16 (both operands quantized on the host): PE rate
is the same 1 cyc/row as fp32r, but the weight loads take the
fast-weight-load path (LDWEIGHTS 97ns vs 187ns, which sets the matmul
cadence), DMA bytes halve, and the end-to-end error stays ~2.3e-3 against
the f32 reference (gate is 2e-2).

DMA-trigger cost (~650ns per dma_start on an engine queue, roughly
independent of size) dominates the startup, so tiles are batched: each
seq chunk of x^T is ONE [128, 8j, C] tile loaded by one dma_start, and W
lives in ONE [128, 8j, 2*D] tile loaded by four k-staged dma_starts
(so the k=0 matmuls only wait for ~0.5MB). Triggers are spread over the
sync/scalar/gpsimd queues so nothing serializes behind the Sync queue,
and the PE stream starts at ~12us and runs gapless (PE idle gaps would
also re-trigger HAM throttling).

Per-core pipeline over seq chunks of C=512:
  DMA x^T chunk tile [128d, 8j, C]
  -> bf16 matmuls hg^T[k] = sum_j W[j,k]^T x^T[j] accumulated in PSUM
  -> ACT sigh = sigmoid(hidden); DVE gh = (hidden + 0.5) max sigh
     (issued between the hidden and gate matmul groups so they overlap
      the gate matmuls)
  -> ACT: a = sigmoid(-gate); DVE: bneg = (a - 1) * gh
  -> DVE: h = scan(a * h_prev) - bneg   (carry chained across chunks)
  -> DMA h^T tile straight to DRAM out^T.
"""

import numpy as np

import concourse.bacc as bacc
import concourse.tile as tile
from concourse import mybir

B, S, D = 8, 4096, 1024
N_CORES = 8
P = 128  # partitions
C = 512  # seq chunk
N_CHUNKS = S // C  # 8
N_DT = D // P  # 8 d-tiles (contraction)
N_KT = D // P  # 8 output channel tiles (hidden dim = D)

F32 = mybir.dt.float32
BF16 = mybir.dt.bfloat16

_COMPILED = {}


def _build():
    nc = bacc.Bacc(
        "TRN2", target_bir_lowering=False, debug=False, num_devices=N_CORES
    )
    xt_d = nc.dram_tensor("xt", [D, S], BF16, kind="ExternalInput").ap()
    w_d = nc.dram_tensor("w", [D, 2 * D], BF16, kind="ExternalInput").ap()
    out_d = nc.dram_tensor("outT", [D, S], F32, kind="ExternalOutput").ap()

    AL = mybir.AluOpType
    SIG = mybir.ActivationFunctionType.Sigmoid

    # DRAM views with the contraction dim split as (j, p)
    xt_v = xt_d.rearrange("(j p) s -> p j s", j=N_DT)  # [128, 8, 4096]
    w_v = w_d.rearrange("(j p) (h c) -> p j h c", j=N_DT, h=2)  # [128,8,2,1024]

    with tile.TileContext(nc) as tc:
        with (
            tc.tile_pool(name="wpool", bufs=1) as wpool,
            tc.tile_pool(name="xtp", bufs=3) as xt_pool,
            tc.tile_pool(name="pw", bufs=3) as pw_pool,
            tc.tile_pool(name="hp", bufs=3) as h_pool,
            tc.tile_pool(name="pshg", bufs=8, space="PSUM") as psum_hg,
        ):
            # one W tile [128, 8j, 2 halves, 1024 cols]
            w_big = wpool.tile([P, N_DT, 2, D], BF16, tag="w", name="w_big")

            def w_dma(eng, k0, k1):
                # cols [k0*P, k1*P) for all j; DMA APs are limited to 3
                # dims, so one trigger per hidden/gate half
                for h in range(2):
                    eng.dma_start(
                        w_big[:, :, h, k0 * P : k1 * P],
                        w_v[:, :, h, k0 * P : k1 * P],
                    )

            def x_tile(name):
                return xt_pool.tile([P, N_DT, C], BF16, tag="xc", name=name)

            def x_dma(eng, t, s0):
                eng.dma_start(t[:], xt_v[:, :, s0 : s0 + C])

            # ---- startup: W streamed back-to-back on sync in k-stages (so
            # the PE never outruns the k-block transfers); chunk-0 x split
            # over scalar+gpsimd; x1 only after x0 so its transfer does not
            # compete with the W stages the PE is about to need.
            w_dma(nc.sync, 0, 1)
            x0 = x_tile("x0")
            nc.scalar.dma_start(x0[:, 0 : N_DT // 2, :], xt_v[:, 0 : N_DT // 2, 0:C])
            nc.gpsimd.dma_start(
                x0[:, N_DT // 2 :, :], xt_v[:, N_DT // 2 :, 0:C]
            )
            # k1 stage j-split so the k=1 hidden matmuls can start on the
            # first half of W while the rest of the stage is still in flight
            # (the PE reaches k=1 ~3.5us after its first matmul)
            J2 = N_DT // 2
            nc.sync.dma_start(w_big[:, 0:J2, 0, P : 2 * P], w_v[:, 0:J2, 0, P : 2 * P])
            nc.sync.dma_start(w_big[:, J2:, 0, P : 2 * P], w_v[:, J2:, 0, P : 2 * P])
            nc.sync.dma_start(w_big[:, 0:J2, 1, P : 2 * P], w_v[:, 0:J2, 1, P : 2 * P])
            nc.sync.dma_start(w_big[:, J2:, 1, P : 2 * P], w_v[:, J2:, 1, P : 2 * P])
            w_dma(nc.sync, 2, 4)
            w_dma(nc.sync, 4, 8)
            # x1 queued on sync BEHIND all W stages: its 1MB transfer must
            # not compete with w_k1/w_k23, which the PE needs first (the
            # cold-start DMA engines only sustain ~200 GB/s).
            x1 = x_tile("x1")
            x_dma(nc.sync, x1, C)

            prev_h = [None] * N_KT
            for sc in range(N_CHUNKS):
                s0 = sc * C
                # ---- load x^T chunk tile (gpsimd queue is idle mid-stream)
                if sc == 0:
                    xts = x0
                elif sc == 1:
                    xts = x1
                else:
                    xts = x_tile(f"xc{sc}")
                    x_dma(nc.gpsimd, xts, s0)
                # ---- per channel-tile k: matmuls + pointwise + scan + store
                for k in range(N_KT):
                    ph = psum_hg.tile([P, C], F32, tag="ph")  # hidden
                    for j in range(N_DT):
                        nc.tensor.matmul(
                            ph[:],
                            w_big[:, j, 0, k * P : (k + 1) * P],
                            xts[:, j, :],
                            start=(j == 0),
                            stop=(j == N_DT - 1),
                        )
                    # hidden-dependent pointwise issued before the gate
                    # matmuls: sigh/gh overlap the PE's gate group.
                    # sigh = sigmoid(hidden)
                    sigh = pw_pool.tile([P, C], F32, tag="sigh")
                    nc.scalar.activation(sigh[:], ph[:], SIG)
                    # g(hidden) = max(hidden + 0.5, sigmoid(hidden))
                    gh = pw_pool.tile([P, C], F32, tag="gh")
                    nc.vector.scalar_tensor_tensor(
                        gh[:], ph[:], 0.5, sigh[:], op0=AL.add, op1=AL.max
                    )
                    pg = psum_hg.tile([P, C], F32, tag="ph")  # gate
                    for j in range(N_DT):
                        nc.tensor.matmul(
                            pg[:],
                            w_big[:, j, 1, k * P : (k + 1) * P],
                            xts[:, j, :],
                            start=(j == 0),
                            stop=(j == N_DT - 1),
                        )
                    # a = sigmoid(-gate) = 1 - z
                    a_t = pw_pool.tile([P, C], F32, tag="a")
                    nc.scalar.activation(a_t[:], pg[:], SIG, scale=-1.0)
                    # bneg = (a - 1) * g = -(z * g)
                    bneg = pw_pool.tile([P, C], F32, tag="bneg")
                    nc.vector.scalar_tensor_tensor(
                        bneg[:], a_t[:], 1.0, gh[:], op0=AL.subtract, op1=AL.mult
                    )
                    # h_t = a_t * h_{t-1} - bneg_t  (linear recurrence)
                    h = h_pool.tile([P, C], F32, tag=f"h{k}")
                    init = 0.0 if prev_h[k] is None else prev_h[k][:, C - 1 : C]
                    if sc == N_CHUNKS - 1 and k >= N_KT - 2:
                        # kernel tail: quarter the final scans/stores so the
                        # stores overlap the remaining scan pieces
                        Q = C // 4
                        carry = init
                        for piece in range(4):
                            lo, hi = piece * Q, (piece + 1) * Q
                            nc.vector.tensor_tensor_scan(
                                h[:, lo:hi], a_t[:, lo:hi], bneg[:, lo:hi],
                                carry, op0=AL.mult, op1=AL.subtract,
                            )
                            nc.sync.dma_start(
                                out_d[k * P : (k + 1) * P, s0 + lo : s0 + hi],
                                h[:, lo:hi],
                            )
                            carry = h[:, hi - 1 : hi]
                    else:
                        nc.vector.tensor_tensor_scan(
                            h[:], a_t[:], bneg[:], init,
                            op0=AL.mult, op1=AL.subtract,
                        )
                        nc.sync.dma_start(
                            out_d[k * P : (k + 1) * P, s0 : s0 + C], h[:]
                        )
                    prev_h[k] = h
    nc.compile()
    return nc


def _get_nc():
    if "bf16" not in _COMPILED:
        _COMPILED["bf16"] = _build()
    return _COMPILED["bf16"]


def _make_in_maps(x: np.ndarray, W_hg: np.ndarray):
    import ml_dtypes

    xb = np.asarray(x, dtype=np.float32).astype(ml_dtypes.bfloat16)
    wb = np.ascontiguousarray(
        np.asarray(W_hg, dtype=np.float32).astype(ml_dtypes.bfloat16)
    )
    return [
        {"xt": np.ascontiguousarray(xb[b].T), "w": wb} for b in range(N_CORES)
    ]


def kernel(x: np.ndarray, W_hg: np.ndarray) -> np.ndarray:
    from concourse.bass_utils import run_bass_kernel_spmd

    assert x.shape == (B, S, D) and W_hg.shape == (D, 2 * D)
    nc = _get_nc()
    in_maps = _make_in_maps(x, W_hg)
    res = run_bass_kernel_spmd(nc, in_maps, list(range(N_CORES)))
    out = np.empty((B, S, D), dtype=np.float32)
    for b in range(N_CORES):
        out[b] = res.results[b]["outT"].T
    return out


# revision 14
# speedup vs baseline: 1.1978x; 1.1978x over previous
"""MinGRU (parallel log-space scan) Trainium2 Bass kernel.

Problem (hardcoded):
    x:    [B=8, S=4096, D=1024] f32
    W_hg: [D=1024, 2*D=2048]    f32
    out:  [B=8, S=4096, D=1024] f32

    hg = x @ W_hg ; hidden, gate = split(hg)
    h_t = (1-z_t) * h_{t-1} + z_t * g(hidden_t),  z = sigmoid(gate),
    g(v) = v + 0.5 if v >= 0 else sigmoid(v)  ==  max(v + 0.5, sigmoid(v))

Sharding: data-parallel over batch, one batch row per NeuronCore (8 cores),
W_hg replicated.

Layout strategy: the scan must run along the free dimension (channels on
partitions), so the device works entirely in the transposed layout
hg^T/h^T = [channels, seq]. The host passes x pre-transposed per batch row
and transposes the returned h^T back, so the device does no layout
conversion at all — the PE runs only the projection matmuls, ACT runs the
sigmoids, and the DVE runs the fused pointwise ops plus the native
tensor_tensor_scan linear recurrence.

The projection runs in bf16 (both operands quantized on the host): PE rate
is the same 1 cyc/row as fp32r, but the weight loads take the
fast-weight-load path (LDWEIGHTS 97ns vs 187ns, which sets the matmul
cadence), DMA bytes halve, and the end-to-end error stays ~2.3e-3 against
the f32 reference (gate is 2e-2).

DMA-trigger cost (~650ns per dma_start on an engine queue, roughly
independent of size) dominates the startup, so tiles are batched: each
seq chunk of x^T is ONE [128, 8j, C] tile loaded by one dma_start, and W
lives in ONE [128, 8j, 2*D] tile loaded by four k-staged dma_starts
(so the k=0 matmuls only wait for ~0.5MB). Triggers are spread over the
sync/scalar/gpsimd queues so nothing serializes behind the Sync queue,
and the PE stream starts at ~12us and runs gapless (PE idle gaps would
also re-trigger HAM throttling).

Per-core pipeline over seq chunks of C=512:
  DMA x^T chunk tile [128d, 8j, C]
  -> bf16 matmuls hg^T[k] = sum_j W[j,k]^T x^T[j] accumulated in PSUM
  -> ACT sigh = sigmoid(hidden); DVE gh = (hidden + 0.5) max sigh
     (issued between the hidden and gate matmul groups so they overlap
      the gate matmuls)
  -> ACT: a = sigmoid(-gate); DVE: bneg = (a - 1) * gh
  -> DVE: h = scan(a * h_prev) - bneg   (carry chained across chunks)
  -> DMA h^T tile straight to DRAM out^T.
"""

import numpy as np

import concourse.bacc as bacc
import concourse.tile as tile
from concourse import mybir

B, S, D = 8, 4096, 1024
N_CORES = 8
P = 128  # partitions
C = 512  # seq chunk
N_CHUNKS = S // C  # 8
N_DT = D // P  # 8 d-tiles (contraction)
N_KT = D // P  # 8 output channel tiles (hidden dim = D)

F32 = mybir.dt.float32
BF16 = mybir.dt.bfloat16

_COMPILED = {}


def _build():
    nc = bacc.Bacc(
        "TRN2", target_bir_lowering=False, debug=False, num_devices=N_CORES
    )
    xt_d = nc.dram_tensor("xt", [D, S], BF16, kind="ExternalInput").ap()
    w_d = nc.dram_tensor("w", [D, 2 * D], BF16, kind="ExternalInput").ap()
    out_d = nc.dram_tensor("outT", [D, S], F32, kind="ExternalOutput").ap()

    AL = mybir.AluOpType
    SIG = mybir.ActivationFunctionType.Sigmoid

    # DRAM views with the contraction dim split as (j, p)
    xt_v = xt_d.rearrange("(j p) s -> p j s", j=N_DT)  # [128, 8, 4096]
    w_v = w_d.rearrange("(j p) (h c) -> p j h c", j=N_DT, h=2)  # [128,8,2,1024]

    with tile.TileContext(nc) as tc:
        with (
            tc.tile_pool(name="wpool", bufs=1) as wpool,
            tc.tile_pool(name="xtp", bufs=3) as xt_pool,
            tc.tile_pool(name="pw", bufs=3) as pw_pool,
            tc.tile_pool(name="hp", bufs=3) as h_pool,
            tc.tile_pool(name="pshg", bufs=8, space="PSUM") as psum_hg,
        ):
            # one W tile [128, 8j, 2 halves, 1024 cols]
            w_big = wpool.tile([P, N_DT, 2, D], BF16, tag="w", name="w_big")

            def w_dma(eng, k0, k1):
                # cols [k0*P, k1*P) for all j; DMA APs are limited to 3
                # dims, so one trigger per hidden/gate half
                for h in range(2):
                    eng.dma_start(
                        w_big[:, :, h, k0 * P : k1 * P],
                        w_v[:, :, h, k0 * P : k1 * P],
                    )

            def x_tile(name):
                return xt_pool.tile([P, N_DT, C], BF16, tag="xc", name=name)

            def x_dma(eng, t, s0):
                eng.dma_start(t[:], xt_v[:, :, s0 : s0 + C])

            # ---- startup: W streamed back-to-back on sync in k-stages (so
            # the PE never outruns the k-block transfers); chunk-0 x split
            # over scalar+gpsimd; x1 only after x0 so its transfer does not
            # compete with the W stages the PE is about to need.
            w_dma(nc.sync, 0, 1)
            x0 = x_tile("x0")
            nc.scalar.dma_start(x0[:, 0 : N_DT // 2, :], xt_v[:, 0 : N_DT // 2, 0:C])
            nc.gpsimd.dma_start(
                x0[:, N_DT // 2 :, :], xt_v[:, N_DT // 2 :, 0:C]
            )
            w_dma(nc.sync, 1, 2)
            w_dma(nc.sync, 2, 4)
            w_dma(nc.sync, 4, 8)
            # x1 queued on sync BEHIND all W stages: its 1MB transfer must
            # not compete with w_k1/w_k23, which the PE needs first (the
            # cold-start DMA engines only sustain ~200 GB/s).
            x1 = x_tile("x1")
            x_dma(nc.sync, x1, C)

            prev_h = [None] * N_KT
            for sc in range(N_CHUNKS):
                s0 = sc * C
                # ---- load x^T chunk tile (gpsimd queue is idle mid-stream)
                if sc == 0:
                    xts = x0
                elif sc == 1:
                    xts = x1
                else:
                    xts = x_tile(f"xc{sc}")
                    x_dma(nc.gpsimd, xts, s0)
                # ---- per channel-tile k: matmuls + pointwise + scan + store
                for k in range(N_KT):
                    ph = psum_hg.tile([P, C], F32, tag="ph")  # hidden
                    for j in range(N_DT):
                        nc.tensor.matmul(
                            ph[:],
                            w_big[:, j, 0, k * P : (k + 1) * P],
                            xts[:, j, :],
                            start=(j == 0),
                            stop=(j == N_DT - 1),
                        )
                    # hidden-dependent pointwise issued before the gate
                    # matmuls: sigh/gh overlap the PE's gate group.
                    # sigh = sigmoid(hidden)
                    sigh = pw_pool.tile([P, C], F32, tag="sigh")
                    nc.scalar.activation(sigh[:], ph[:], SIG)
                    # g(hidden) = max(hidden + 0.5, sigmoid(hidden))
                    gh = pw_pool.tile([P, C], F32, tag="gh")
                    nc.vector.scalar_tensor_tensor(
                        gh[:], ph[:], 0.5, sigh[:], op0=AL.add, op1=AL.max
                    )
                    pg = psum_hg.tile([P, C], F32, tag="ph")  # gate
                    for j in range(N_DT):
                        nc.tensor.matmul(
                            pg[:],
                            w_big[:, j, 1, k * P : (k + 1) * P],
                            xts[:, j, :],
                            start=(j == 0),
                            stop=(j == N_DT - 1),
                        )
                    # a = sigmoid(-gate) = 1 - z
                    a_t = pw_pool.tile([P, C], F32, tag="a")
                    nc.scalar.activation(a_t[:], pg[:], SIG, scale=-1.0)
                    # bneg = (a - 1) * g = -(z * g)
                    bneg = pw_pool.tile([P, C], F32, tag="bneg")
                    nc.vector.scalar_tensor_tensor(
                        bneg[:], a_t[:], 1.0, gh[:], op0=AL.subtract, op1=AL.mult
                    )
                    # h_t = a_t * h_{t-1} - bneg_t  (linear recurrence)
                    h = h_pool.tile([P, C], F32, tag=f"h{k}")
                    init = 0.0 if prev_h[k] is None else prev_h[k][:, C - 1 : C]
                    if sc == N_CHUNKS - 1 and k >= N_KT - 2:
                        # kernel tail: quarter the final scans/stores so the
                        # stores overlap the remaining scan pieces
                        Q = C // 4
                        carry = init
                        for piece in range(4):
                            lo, hi = piece * Q, (piece + 1) * Q
                            nc.vector.tensor_tensor_scan(
                                h[:, lo:hi], a_t[:, lo:hi], bneg[:, lo:hi],
                                carry, op0=AL.mult, op1=AL.subtract,
                            )
                            nc.sync.dma_start(
                                out_d[k * P : (k + 1) * P, s0 + lo : s0 + hi],
                                h[:, lo:hi],
                            )
                            carry = h[:, hi - 1 : hi]
                    else:
                        nc.vector.tensor_tensor_scan(
                            h[:], a_t[:], bneg[:], init,
                            op0=AL.mult, op1=AL.subtract,
                        )
                        nc.sync.dma_start(
                            out_d[k * P : (k + 1) * P, s0 : s0 + C], h[:]
                        )
                    prev_h[k] = h
    nc.compile()
    return nc


def _get_nc():
    if "bf16" not in _COMPILED:
        _COMPILED["bf16"] = _build()
    return _COMPILED["bf16"]


def _make_in_maps(x: np.ndarray, W_hg: np.ndarray):
    import ml_dtypes

    xb = np.asarray(x, dtype=np.float32).astype(ml_dtypes.bfloat16)
    wb = np.ascontiguousarray(
        np.asarray(W_hg, dtype=np.float32).astype(ml_dtypes.bfloat16)
    )
    return [
        {"xt": np.ascontiguousarray(xb[b].T), "w": wb} for b in range(N_CORES)
    ]


def kernel(x: np.ndarray, W_hg: np.ndarray) -> np.ndarray:
    from concourse.bass_utils import run_bass_kernel_spmd

    assert x.shape == (B, S, D) and W_hg.shape == (D, 2 * D)
    nc = _get_nc()
    in_maps = _make_in_maps(x, W_hg)
    res = run_bass_kernel_spmd(nc, in_maps, list(range(N_CORES)))
    out = np.empty((B, S, D), dtype=np.float32)
    for b in range(N_CORES):
        out[b] = res.results[b]["outT"].T
    return out
